# revision 44
# baseline (speedup 1.0000x reference)
"""Depthwise causal conv1d kernel for Trainium2 (8 NeuronCores, SPMD).

Problem: x [B=8, T=4096, C=512] f32, weight [C=512, K=4] f32.
out[b, t, c] = sum_k weight[c, k] * x[b, t - 3 + k, c]   (causal, zero-pad)

Strategy:
  - Data-parallel over batch: core b handles x[b].
  - Host-side layout: each core's input is channels-first x[b].T padded
    with K-1 = 3 leading zeros along time -> [C=512, T+3=4099], cast to
    fp16, so the device kernel sees contiguous time on the free axis and
    channels on partitions. fp16 halves the input traffic (the kernel is
    HBM-bound) and its 11-bit mantissa keeps the conv's error at the
    ~2^-11 level; accumulation stays fp32 in PSUM.
  - Device: an 8 KB fp16 weight-column table lands first; GpSimd expands
    it into 16 diag(weight[:, k]) [128x128] stationary matrices while the
    x chunks stream in. For each 128-channel chunk the 4-tap conv is 4
    accumulating TensorE matmuls (stationary diag, moving = shifted x
    views) at full single-pass PE rate. PSUM results are cast to fp16
    during the PSUM->SBUF copy (VectorE even chunks, ScalarE odd) and
    shipped as soon as each half-chunk's copies land; the host upcasts
    to fp32. ~8.4 MB HBM traffic per core; the kernel ends PE-bound at
    ~31 us of matmul inside a ~47 us exec window.
"""

import numpy as np

B, T, C, K = 8, 4096, 512, 4
P = 128  # partitions
NCHUNK = C // P  # 4 channel chunks
TJ = 512  # time-tile (free dim) per matmul; one PSUM bank
NJ = T // TJ  # 8 time tiles per chunk
TP = T + K - 1  # padded time = 4099
NW = NCHUNK * K  # 16 (chunk, tap) pairs

_compiled = None


def _build():
    import concourse.bacc as bacc
    import concourse.bass as bass
    import concourse.mybir as mybir
    from concourse.tile import TileContext

    f32 = mybir.dt.float32
    f16 = mybir.dt.float16
    nc = bacc.Bacc(enable_partition_id=False)

    wt_d = nc.declare_dram_parameter("wt", [P, NW], f16, isOutput=False)
    xw_d = nc.declare_dram_parameter("xw", [P, NCHUNK * TP], f16, isOutput=False)
    out_d = nc.declare_dram_parameter("out", [C, T], f16, isOutput=True)

    with TileContext(nc) as tc:
        with (
            tc.tile_pool(name="xpool", bufs=1) as xpool,
            tc.tile_pool(name="wpool", bufs=1) as wpool,
            tc.tile_pool(name="opool", bufs=4) as opool,
            tc.tile_pool(name="ppool", bufs=4, space="PSUM") as ppool,
        ):
            # weight table first: tiny DMA, expansion overlaps the x loads
            wcol = wpool.tile([P, NW], f16, tag="wcol")
            nc.sync.dma_start(out=wcol, in_=wt_d[:, :])
            # chunk0's x split in two so its first time-tiles land sooner
            xts = []
            xt0 = xpool.tile([P, TP], f16, name="xt0", tag="xt0")
            h0 = TJ + K - 1  # 515 cols: j-tile 0 + halo
            h1 = NJ // 2 * TJ + K - 1  # j-tiles 1..3
            nc.sync.dma_start(out=xt0[:, :h0], in_=xw_d[:, 0:h0])
            nc.sync.dma_start(out=xt0[:, h0:h1], in_=xw_d[:, h0:h1])
            nc.sync.dma_start(out=xt0[:, h1:], in_=xw_d[:, h1:TP])
            xts.append(xt0)
            for c in range(1, NCHUNK):
                xt = xpool.tile([P, TP], f16, name=f"xt{c}", tag=f"xt{c}")
                nc.sync.dma_start(out=xt, in_=xw_d[:, c * TP : (c + 1) * TP])
                xts.append(xt)

            # expand wcol into per-chunk diag stationary tiles on GpSimd;
            # separate tiles so chunk0's matmuls wait only on its own 4
            wts = []
            for c in range(NCHUNK):
                wt = wpool.tile([P, K * P], f16, name=f"wd{c}", tag=f"wd{c}")
                for k in range(K):
                    idx = c * K + k
                    wsrc = bass.AP(
                        wcol.tensor, wcol.offset + idx, [[NW, P], [0, P]]
                    )
                    nc.gpsimd.affine_select(
                        out=wt[:, k * P : (k + 1) * P],
                        in_=wsrc,
                        compare_op=mybir.AluOpType.is_equal,
                        fill=0.0,
                        base=0,
                        # iota[p, i] = p - i; == 0 on the diagonal
                        pattern=[[-1, P]],
                        channel_multiplier=1,
                    )
                wts.append(wt)

            TJ2 = 2 * TJ  # j-tile pair: one 2-bank PSUM tile, one copy
            for chunk in range(NCHUNK):
                xv = xts[chunk]
                wt = wts[chunk]
                ot = opool.tile([P, T], f16, tag="ot")
                for jj in range(NJ // 2):
                    pt = ppool.tile([P, TJ2], f32, name="pt", tag="pt")
                    for half in range(2):
                        j = 2 * jj + half
                        for k in range(K):
                            nc.tensor.matmul(
                                pt[:, half * TJ : (half + 1) * TJ],
                                wt[:, k * P : (k + 1) * P],
                                xv[:, j * TJ + k : j * TJ + k + TJ],
                                start=(k == 0),
                                stop=(k == K - 1),
                            )
                    dst = ot[:, jj * TJ2 : (jj + 1) * TJ2]
                    # DVE takes the odd chunks so the LAST chunk's tail
                    # copy is the faster engine (DVE cast-copy ~0.66 us
                    # vs ACT ~1.0 us)
                    if chunk % 2 == 0:
                        nc.scalar.copy(dst, pt)
                    else:
                        nc.vector.tensor_copy(dst, pt)
                    # ship output as soon as its copies land; the last
                    # chunk goes out in pair-pieces to shorten the tail
                    last = chunk == NCHUNK - 1
                    if last and jj >= NJ // 4:
                        lo_c = jj * TJ2
                        nc.sync.dma_start(
                            out=out_d[chunk * P : (chunk + 1) * P, lo_c : lo_c + TJ2],
                            in_=ot[:, lo_c : lo_c + TJ2],
                        )
                    elif jj == NJ // 4 - 1 or jj == NJ // 2 - 1:
                        half_c = 0 if jj < NJ // 4 else NJ // 4 * TJ2
                        nc.sync.dma_start(
                            out=out_d[
                                chunk * P : (chunk + 1) * P,
                                half_c : half_c + NJ // 4 * TJ2,
                            ],
                            in_=ot[:, half_c : half_c + NJ // 4 * TJ2],
                        )

    nc.compile()
    return nc


def _prep_inputs(x: np.ndarray, weight: np.ndarray):
    # wcol[p, chunk*K + k] = weight[chunk*P + p, k]
    wcol = np.ascontiguousarray(
        weight.reshape(NCHUNK, P, K).transpose(1, 0, 2).reshape(P, NW)
    ).astype(np.float16)
    xs = []
    for b in range(B):
        xp = np.zeros((C, TP), dtype=np.float32)
        xp[:, K - 1 :] = x[b].T  # [512, 4099], 3 leading zeros
        xw = np.ascontiguousarray(
            xp.reshape(NCHUNK, P, TP).transpose(1, 0, 2).reshape(P, NCHUNK * TP)
        ).astype(np.float16)
        xs.append(xw)
    return xs, wcol


def _ensure_axon_hooks():
    """This image's antenv package lacks axon_hooks; synthesize it so a
    trace=True / BASS_TRACE run of run_bass_kernel_spmd can profile
    instead of crashing on import."""
    import sys
    import types

    if "antenv.axon_hooks" in sys.modules:
        return
    mod = types.ModuleType("antenv.axon_hooks")
    state = {"hook": None}
    mod.set_axon_ntff_profile_hook = lambda h: state.__setitem__("hook", h)
    mod.get_axon_ntff_profile_hook = lambda: state["hook"]
    sys.modules["antenv.axon_hooks"] = mod
    try:
        if "/root/.axon_site" not in sys.path:
            sys.path.insert(0, "/root/.axon_site")
        from trn_agent_boot.trn_boot import _ntff_profile_via_ctypes

        mod.set_axon_ntff_profile_hook(
            _ntff_profile_via_ctypes("/opt/axon/libaxon_pjrt.so")
        )
    except Exception:
        pass  # hook stays None; concourse degrades to no-trace


def kernel(x: np.ndarray, weight: np.ndarray) -> np.ndarray:
    global _compiled
    _ensure_axon_hooks()
    from concourse import bass_utils

    x = np.ascontiguousarray(x, dtype=np.float32)
    weight = np.ascontiguousarray(weight, dtype=np.float32)

    if _compiled is None:
        _compiled = _build()
    nc = _compiled

    xs, wcol = _prep_inputs(x, weight)
    in_maps = [{"xw": xs[b], "wt": wcol} for b in range(B)]
    res = bass_utils.run_bass_kernel_spmd(nc, in_maps, core_ids=list(range(B)))

    out = np.empty((B, T, C), dtype=np.float32)
    for b in range(B):
        out[b] = np.asarray(res.results[b]["out"]).astype(np.float32).T
    return out


# revision 45
# speedup vs baseline: 1.0099x; 1.0099x over previous
"""Depthwise causal conv1d kernel for Trainium2 (8 NeuronCores, SPMD).

Problem: x [B=8, T=4096, C=512] f32, weight [C=512, K=4] f32.
out[b, t, c] = sum_k weight[c, k] * x[b, t - 3 + k, c]   (causal, zero-pad)

Strategy:
  - Data-parallel over batch: core b handles x[b].
  - Host-side layout: each core's input is channels-first x[b].T padded
    with K-1 = 3 leading zeros along time -> [C=512, T+3=4099], cast to
    fp16, so the device kernel sees contiguous time on the free axis and
    channels on partitions. fp16 halves the input traffic (the kernel is
    HBM-bound) and its 11-bit mantissa keeps the conv's error at the
    ~2^-11 level; accumulation stays fp32 in PSUM.
  - Device: an 8 KB fp16 weight-column table lands first; GpSimd expands
    it into 16 diag(weight[:, k]) [128x128] stationary matrices while the
    x chunks stream in. For each 128-channel chunk the 4-tap conv is 4
    accumulating TensorE matmuls (stationary diag, moving = shifted x
    views) at full single-pass PE rate, paired into 2-bank PSUM tiles.
    PSUM results are cast to fp16 during the PSUM->SBUF copy (ScalarE
    even chunks, VectorE odd — so the last chunk's tail copy is on the
    faster engine) and shipped as soon as each half-chunk's copies land;
    the host upcasts to fp32. ~8.4 MB HBM traffic per core; the kernel
    ends PE-bound at ~29 us of matmul inside a ~46.5 us exec window.
"""

import numpy as np

B, T, C, K = 8, 4096, 512, 4
P = 128  # partitions
NCHUNK = C // P  # 4 channel chunks
TJ = 512  # time-tile (free dim) per matmul; one PSUM bank
NJ = T // TJ  # 8 time tiles per chunk
TP = T + K - 1  # padded time = 4099
NW = NCHUNK * K  # 16 (chunk, tap) pairs

_compiled = None


def _build():
    import concourse.bacc as bacc
    import concourse.bass as bass
    import concourse.mybir as mybir
    from concourse.tile import TileContext

    f32 = mybir.dt.float32
    f16 = mybir.dt.float16
    nc = bacc.Bacc(enable_partition_id=False)

    wt_d = nc.declare_dram_parameter("wt", [P, NW], f16, isOutput=False)
    xw_d = nc.declare_dram_parameter("xw", [P, NCHUNK * TP], f16, isOutput=False)
    out_d = nc.declare_dram_parameter("out", [C, T], f16, isOutput=True)

    with TileContext(nc) as tc:
        with (
            tc.tile_pool(name="xpool", bufs=1) as xpool,
            tc.tile_pool(name="wpool", bufs=1) as wpool,
            tc.tile_pool(name="opool", bufs=4) as opool,
            tc.tile_pool(name="ppool", bufs=4, space="PSUM") as ppool,
        ):
            # weight table first: tiny DMA, expansion overlaps the x loads
            wcol = wpool.tile([P, NW], f16, tag="wcol")
            nc.sync.dma_start(out=wcol, in_=wt_d[:, :])
            # chunk0's x split in two so its first time-tiles land sooner
            xts = []
            xt0 = xpool.tile([P, TP], f16, name="xt0", tag="xt0")
            h0 = TJ + K - 1  # 515 cols: j-tile 0 + halo
            h1 = NJ // 2 * TJ + K - 1  # j-tiles 1..3
            nc.sync.dma_start(out=xt0[:, :h0], in_=xw_d[:, 0:h0])
            nc.sync.dma_start(out=xt0[:, h0:h1], in_=xw_d[:, h0:h1])
            nc.sync.dma_start(out=xt0[:, h1:], in_=xw_d[:, h1:TP])
            xts.append(xt0)
            for c in range(1, NCHUNK):
                xt = xpool.tile([P, TP], f16, name=f"xt{c}", tag=f"xt{c}")
                nc.sync.dma_start(out=xt, in_=xw_d[:, c * TP : (c + 1) * TP])
                xts.append(xt)

            # expand wcol into per-chunk diag stationary tiles on GpSimd;
            # separate tiles so chunk0's matmuls wait only on its own 4
            wts = []
            for c in range(NCHUNK):
                wt = wpool.tile([P, K * P], f16, name=f"wd{c}", tag=f"wd{c}")
                for k in range(K):
                    idx = c * K + k
                    wsrc = bass.AP(
                        wcol.tensor, wcol.offset + idx, [[NW, P], [0, P]]
                    )
                    nc.gpsimd.affine_select(
                        out=wt[:, k * P : (k + 1) * P],
                        in_=wsrc,
                        compare_op=mybir.AluOpType.is_equal,
                        fill=0.0,
                        base=0,
                        # iota[p, i] = p - i; == 0 on the diagonal
                        pattern=[[-1, P]],
                        channel_multiplier=1,
                    )
                wts.append(wt)

            TJ2 = 2 * TJ  # j-tile pair: one 2-bank PSUM tile, one copy
            for chunk in range(NCHUNK):
                xv = xts[chunk]
                wt = wts[chunk]
                ot = opool.tile([P, T], f16, tag="ot")
                for jj in range(NJ // 2):
                    pt = ppool.tile([P, TJ2], f32, name="pt", tag="pt")
                    for half in range(2):
                        j = 2 * jj + half
                        for k in range(K):
                            nc.tensor.matmul(
                                pt[:, half * TJ : (half + 1) * TJ],
                                wt[:, k * P : (k + 1) * P],
                                xv[:, j * TJ + k : j * TJ + k + TJ],
                                start=(k == 0),
                                stop=(k == K - 1),
                            )
                    dst = ot[:, jj * TJ2 : (jj + 1) * TJ2]
                    # DVE takes the odd chunks so the LAST chunk's tail
                    # copy is the faster engine (DVE cast-copy ~0.66 us
                    # vs ACT ~1.0 us)
                    if chunk % 2 == 0:
                        nc.scalar.copy(dst, pt)
                    else:
                        nc.vector.tensor_copy(dst, pt)
                    # ship output as soon as its copies land; the last
                    # chunk goes out in pair-pieces to shorten the tail
                    last = chunk == NCHUNK - 1
                    if last and jj >= NJ // 4:
                        lo_c = jj * TJ2
                        nc.sync.dma_start(
                            out=out_d[chunk * P : (chunk + 1) * P, lo_c : lo_c + TJ2],
                            in_=ot[:, lo_c : lo_c + TJ2],
                        )
                    elif jj == NJ // 4 - 1 or jj == NJ // 2 - 1:
                        half_c = 0 if jj < NJ // 4 else NJ // 4 * TJ2
                        nc.sync.dma_start(
                            out=out_d[
                                chunk * P : (chunk + 1) * P,
                                half_c : half_c + NJ // 4 * TJ2,
                            ],
                            in_=ot[:, half_c : half_c + NJ // 4 * TJ2],
                        )

    nc.compile()
    return nc


def _prep_inputs(x: np.ndarray, weight: np.ndarray):
    # wcol[p, chunk*K + k] = weight[chunk*P + p, k]
    wcol = np.ascontiguousarray(
        weight.reshape(NCHUNK, P, K).transpose(1, 0, 2).reshape(P, NW)
    ).astype(np.float16)
    xs = []
    for b in range(B):
        xp = np.zeros((C, TP), dtype=np.float32)
        xp[:, K - 1 :] = x[b].T  # [512, 4099], 3 leading zeros
        xw = np.ascontiguousarray(
            xp.reshape(NCHUNK, P, TP).transpose(1, 0, 2).reshape(P, NCHUNK * TP)
        ).astype(np.float16)
        xs.append(xw)
    return xs, wcol


def _ensure_axon_hooks():
    """This image's antenv package lacks axon_hooks; synthesize it so a
    trace=True / BASS_TRACE run of run_bass_kernel_spmd can profile
    instead of crashing on import."""
    import sys
    import types

    if "antenv.axon_hooks" in sys.modules:
        return
    mod = types.ModuleType("antenv.axon_hooks")
    state = {"hook": None}
    mod.set_axon_ntff_profile_hook = lambda h: state.__setitem__("hook", h)
    mod.get_axon_ntff_profile_hook = lambda: state["hook"]
    sys.modules["antenv.axon_hooks"] = mod
    try:
        if "/root/.axon_site" not in sys.path:
            sys.path.insert(0, "/root/.axon_site")
        from trn_agent_boot.trn_boot import _ntff_profile_via_ctypes

        mod.set_axon_ntff_profile_hook(
            _ntff_profile_via_ctypes("/opt/axon/libaxon_pjrt.so")
        )
    except Exception:
        pass  # hook stays None; concourse degrades to no-trace


def kernel(x: np.ndarray, weight: np.ndarray) -> np.ndarray:
    global _compiled
    _ensure_axon_hooks()
    from concourse import bass_utils

    x = np.ascontiguousarray(x, dtype=np.float32)
    weight = np.ascontiguousarray(weight, dtype=np.float32)

    if _compiled is None:
        _compiled = _build()
    nc = _compiled

    xs, wcol = _prep_inputs(x, weight)
    in_maps = [{"xw": xs[b], "wt": wcol} for b in range(B)]
    res = bass_utils.run_bass_kernel_spmd(nc, in_maps, core_ids=list(range(B)))

    out = np.empty((B, T, C), dtype=np.float32)
    for b in range(B):
        out[b] = np.asarray(res.results[b]["out"]).astype(np.float32).T
    return out
